# revision 1
# baseline (speedup 1.0000x reference)
"""Trainium2 Bass kernel for nn_Augmenter (color jitter + translate + cutout).

Contract: kernel(**inputs) takes FULL unsharded numpy inputs
(imgs [128,3,256,256] f32, br/sat/con [128,1,1,1] f32,
 tx/ty/cx/cy [128,1,1] i32) and returns the FULL output [128,3,256,256] f32.

Internally: shard batch over 8 NeuronCores (16 images each), run one SPMD
Bass/Tile kernel via run_bass_kernel_spmd, reassemble on host.

Math (per image, derived from the reference):
  b = br-0.5, s = 2*sat, c = con+0.5
  color:  x3 = A*x + Bp*MC + D
          A  = c*s
          Bp = c*(1-s)/3          (MC = sum over the 3 channels of x)
          D  = (1-c)*m0 + b       (m0 = mean over all pixels+channels of x)
  translate by (txs,tys) = (tx-32, ty-32) with zero fill
  cutout: zero rows [max(0,cx-64), min(255,cx+63)] x cols [..cy..]

Implementation notes:
  * The translation (rows AND cols) is done by ONE flat dynamic-offset DMA
    store per plane: writing the color-transformed plane at flat offset
    -(txs*256+tys) relative to a fixed extraction window inside a padded
    output slot. Column wrap-around garbage is zeroed on-chip by a
    column-validity vector folded into the mask; uncovered head/tail rows
    rely on the harness pre-zeroing ExternalOutput buffers (the native
    run_bass_kernel_spmd path documents this; bass2jax donates zero buffers).
  * mask'[r,s] = rc[r]*ccs[s] - cvs[s]  (= -mask) is built on the otherwise
    idle TensorEngine as accumulated rank-1 matmuls into PSUM.
    The sign is folded into negated A/Bp/D so out = (-x3)*mask' = x3*mask.
  * SBUF plane layout: [128 partitions, 512 free]; partition p holds image
    rows 2p and 2p+1 (flat row-major <-> (p, free) is the identity), so both
    load and store DMAs are fully contiguous (2KB per partition).
"""

import numpy as np

import concourse.bacc as bacc
import concourse.bass as bass
import concourse.mybir as mybir
import concourse.tile as tile
from concourse.bass_isa import ReduceOp
from concourse.bass_utils import run_bass_kernel_spmd

F32 = mybir.dt.float32
I32 = mybir.dt.int32
OP = mybir.AluOpType
AF = mybir.ActivationFunctionType

N_CORES = 8
B_FULL = 128
IMGS_PER_CORE = B_FULL // N_CORES  # 16
C, H, W = 3, 256, 256
PLANE = H * W  # 65536

# Padded output slot geometry. Dynamic store offset within a slot is
# off = MARG - s0, s0 = txs*256 + tys in [-8224, 8224], MARG = 8448.
# off in [224, 16672]; the write occupies [off, off+PLANE) of the slot.
MARG = 8448
SLOT = PLANE + MARG  # 73984 stride; margins shared between neighbours
OFF_MIN, OFF_MAX = 224, 16672


def _build_kernel(n_imgs: int, repeat: int = 1):
    """Build + compile the per-core SPMD program.

    repeat > 1 re-emits the per-image pipeline (identical work+writes) for
    amortized wall-clock timing; output is unchanged.
    """
    nc = bacc.Bacc(
        "TRN2",
        target_bir_lowering=False,
        debug=False,
        enable_asserts=False,
        num_devices=N_CORES,
    )
    n_planes = n_imgs * C
    out_flat = (n_planes - 1) * SLOT + OFF_MAX + PLANE

    imgs_t = nc.dram_tensor("imgs", [n_planes, PLANE], F32, kind="ExternalInput")
    # params twice: row layout [1, 8*n] and column layout [n, 8]
    prmr_t = nc.dram_tensor("prmr", [1, 8 * n_imgs], F32, kind="ExternalInput")
    prmc_t = nc.dram_tensor("prmc", [n_imgs, 8], F32, kind="ExternalInput")
    out_t = nc.dram_tensor("out", [out_flat], F32, kind="ExternalOutput")
    imgs = imgs_t.ap()
    prmr = prmr_t.ap()
    prmc = prmc_t.ap()
    out = out_t.ap()

    with tile.TileContext(nc) as tc:
        with (
            tc.tile_pool(name="const", bufs=1) as cpool,
            tc.tile_pool(name="xin", bufs=9) as xpool,
            tc.tile_pool(name="tsum", bufs=2) as tpool,
            tc.tile_pool(name="mc", bufs=3) as mcpool,
            tc.tile_pool(name="tmp", bufs=3) as tmppool,
            tc.tile_pool(name="msk", bufs=3) as mskpool,
            tc.tile_pool(name="yy", bufs=3) as ypool,
            tc.tile_pool(name="oo", bufs=4) as opool,
            tc.tile_pool(name="sm", bufs=8) as smpool,
            tc.tile_pool(name="vr", bufs=6) as vrpool,
            tc.tile_pool(name="ps", bufs=2, space="PSUM") as pspool,
        ):
            V = nc.vector

            # ---------------- one-time setup ----------------
            io_i = cpool.tile([n_imgs, 256], I32)
            nc.gpsimd.iota(io_i, pattern=[[1, 256]], base=0, channel_multiplier=0)
            IO = cpool.tile([n_imgs, 256], F32)
            V.tensor_copy(IO, io_i)

            ONES = cpool.tile([1, 128], F32)
            V.memset(ONES, 1.0)

            # static scatter-offset skeleton: 512*p + SLOT*c  (c = channel)
            # (iota steps are int16-limited, so compose from two small iotas)
            ic3_i = cpool.tile([128, 3], I32)
            nc.gpsimd.iota(ic3_i, pattern=[[1, 3]], base=0, channel_multiplier=0)
            ip_i = cpool.tile([128, 1], I32)
            nc.gpsimd.iota(ip_i, pattern=[[1, 1]], base=0, channel_multiplier=512)
            IC3f = cpool.tile([128, 3], F32)
            V.tensor_copy(IC3f, ic3_i)
            IPf = cpool.tile([128, 1], F32)
            V.tensor_copy(IPf, ip_i)
            ICSf = cpool.tile([128, 3], F32)
            V.tensor_scalar(ICSf, IC3f, float(SLOT), IPf[:, 0:1], OP.mult, OP.add)

            # row-layout params [1, 8*n]: slot g*n_imgs + i = param g of image i
            Pr = cpool.tile([1, 8 * n_imgs], F32)
            nc.scalar.dma_start(Pr, prmr)
            n = n_imgs
            BRr, CONr = Pr[:, 0 * n : 1 * n], Pr[:, 2 * n : 3 * n]
            SATr = Pr[:, 1 * n : 2 * n]
            TXr, TYr = Pr[:, 3 * n : 4 * n], Pr[:, 4 * n : 5 * n]

            # column-layout params [n, 8]
            Pc = cpool.tile([n_imgs, 8], F32)
            nc.scalar.dma_start(Pc, prmc)
            TXc, TYc = Pc[:, 3:4], Pc[:, 4:5]
            CXc, CYc = Pc[:, 5:6], Pc[:, 6:7]

            # --- row-layout crunch: negA/negBp/offbase (-> P3), ep, bpp ---
            # P3 row: [1, 4*n]; image i slots [4i,4i+4) = negA, negBp, negD, offbase
            P3 = cpool.tile([1, 4 * n_imgs], F32)
            negA = P3[:, 0 : 4 * n : 4]
            negBp = P3[:, 1 : 4 * n : 4]
            offb = P3[:, 3 : 4 * n : 4]
            ROW = cpool.tile([1, 4 * n_imgs], F32)
            cf = ROW[:, 0 * n : 1 * n]
            ep = ROW[:, 1 * n : 2 * n]
            bpp = ROW[:, 2 * n : 3 * n]
            rt = ROW[:, 3 * n : 4 * n]

            V.tensor_scalar(cf, CONr, 1.0, 0.5, OP.mult, OP.add)
            V.tensor_scalar(ep, cf, 1.0 / 196608.0, -1.0 / 196608.0, OP.mult, OP.add)
            V.tensor_scalar(bpp, BRr, -1.0, 0.5, OP.mult, OP.add)
            V.tensor_scalar(rt, SATr, 2.0, None, OP.mult)
            V.tensor_tensor(rt, cf, rt, OP.mult)  # A = c*2sat
            V.tensor_scalar(negA, rt, -1.0, None, OP.mult)
            V.tensor_tensor(rt, rt, cf, OP.subtract)  # A - c
            V.tensor_scalar(negBp, rt, 1.0 / 3.0, None, OP.mult)

            # scatter offset base: MARG - s0 = 16672 - 256*tx - ty
            V.tensor_scalar(offb, TXr, -256.0, 16672.0, OP.mult, OP.add)
            V.tensor_tensor(offb, offb, TYr, OP.subtract)

            # --- column-layout crunch + batched mask vectors [n, 256] ---
            COL = cpool.tile([n_imgs, 6], F32)
            txs_c = COL[:, 0:1]
            tys_c = COL[:, 1:2]
            lo = COL[:, 2:3]
            hi = COL[:, 3:4]
            V.tensor_scalar(txs_c, TXc, 32.0, None, OP.subtract)
            V.tensor_scalar(tys_c, TYc, 32.0, None, OP.subtract)

            RC = cpool.tile([n_imgs, 256], F32)   # row in (shifted) cut range
            CCS = cpool.tile([n_imgs, 256], F32)  # col in (shifted) cut range
            NCV = cpool.tile([n_imgs, 256], F32)  # -(col valid)
            e1 = cpool.tile([n_imgs, 256], F32)

            # rows: lo_x = max(0,cx-64)+txs ; hi_x = min(255,cx+63)+txs
            V.tensor_scalar(lo, CXc, 64.0, 0.0, OP.subtract, OP.max)
            V.tensor_tensor(lo, lo, txs_c, OP.add)
            V.tensor_scalar(hi, CXc, 63.0, 255.0, OP.add, OP.min)
            V.tensor_tensor(hi, hi, txs_c, OP.add)
            V.tensor_scalar(e1, IO, hi, None, OP.is_le)
            V.scalar_tensor_tensor(RC, IO, lo, e1, OP.is_ge, OP.logical_and)

            # cols: lo_y = max(0,cy-64)+tys ; hi_y = min(255,cy+63)+tys
            V.tensor_scalar(lo, CYc, 64.0, 0.0, OP.subtract, OP.max)
            V.tensor_tensor(lo, lo, tys_c, OP.add)
            V.tensor_scalar(hi, CYc, 63.0, 255.0, OP.add, OP.min)
            V.tensor_tensor(hi, hi, tys_c, OP.add)
            V.tensor_scalar(e1, IO, hi, None, OP.is_le)
            V.scalar_tensor_tensor(CCS, IO, lo, e1, OP.is_ge, OP.logical_and)

            # -(tys <= s < tys+256)
            V.tensor_scalar(hi, tys_c, 256.0, None, OP.add)
            V.tensor_scalar(e1, IO, hi, None, OP.is_lt)
            V.scalar_tensor_tensor(NCV, IO, tys_c, e1, OP.is_ge, OP.logical_and)
            V.tensor_scalar(NCV, NCV, -1.0, None, OP.mult)

            # ---------------- per-image pipeline ----------------
            for rep in range(repeat):
              for i in range(n_imgs):
                  x = [
                      xpool.tile([128, 512], F32, tag="x", name=f"x{i}_{c}")
                      for c in range(C)
                  ]
                  for c in range(C):
                      nc.scalar.dma_start(
                          x[c], imgs[i * C + c].rearrange("(p f) -> p f", p=128)
                      )

                  t = tpool.tile([128, 512], F32, tag="t")
                  V.tensor_tensor(t, x[0], x[1], OP.add)
                  MC = mcpool.tile([128, 512], F32, tag="mc")
                  mcp = smpool.tile([128, 1], F32, tag="mcp")
                  V.scalar_tensor_tensor(MC, t, 1.0, x[2], OP.mult, OP.add, accum_out=mcp)
                  m0r = smpool.tile([128, 1], F32, tag="m0r")
                  nc.gpsimd.partition_all_reduce(m0r, mcp, 128, ReduceOp.add)
                  # negD = ep*SUM + bpp  -> P3[0, 4i+2]
                  V.scalar_tensor_tensor(
                      P3[:, 4 * i + 2 : 4 * i + 3],
                      m0r[0:1, 0:1],
                      ep[:, i : i + 1],
                      bpp[:, i : i + 1],
                      OP.mult,
                      OP.add,
                  )
                  Sb = smpool.tile([128, 4], F32, tag="sb")
                  nc.gpsimd.partition_broadcast(Sb, P3[:, 4 * i : 4 * i + 4])

                  # scatter offsets: 512*p + SLOT*c + offbase + 3*i*SLOT
                  offtf = smpool.tile([128, 3], F32, tag="offtf")
                  V.tensor_scalar(
                      offtf, ICSf, Sb[:, 3:4], float(3 * i * SLOT), OP.add, OP.add
                  )
                  offt = smpool.tile([128, 3], I32, tag="offt")
                  V.tensor_copy(offt, offtf)

                  # tmp' = negBp*MC + negD   (ScalarE)
                  tmp = tmppool.tile([128, 512], F32, tag="tmp")
                  nc.scalar.activation(
                      tmp, MC, AF.Identity, bias=Sb[:, 2:3], scale=Sb[:, 1:2]
                  )

                  # stage this image's mask vectors at partition 0 (tiny DMAs)
                  rcr = vrpool.tile([1, 256], F32, tag="rcr")
                  ccr = vrpool.tile([1, 256], F32, tag="ccr")
                  nvr = vrpool.tile([1, 256], F32, tag="nvr")
                  nc.sync.dma_start(rcr, RC[i : i + 1, :])
                  nc.sync.dma_start(ccr, CCS[i : i + 1, :])
                  nc.sync.dma_start(nvr, NCV[i : i + 1, :])

                  # mask' = rc x ccs - 1 x cvs   (PE, rank-2 into PSUM)
                  pm = pspool.tile([128, 512], F32, tag="pm")
                  for b in range(2):
                      half = pm[:, b * 256 : (b + 1) * 256]
                      nc.tensor.matmul(
                          half,
                          lhsT=rcr[:, b : 256 : 2],  # rc[2p+b] over p
                          rhs=ccr,
                          start=True,
                          stop=False,
                      )
                      nc.tensor.matmul(half, lhsT=ONES, rhs=nvr, start=False, stop=True)
                  msk = mskpool.tile([128, 512], F32, tag="msk")
                  nc.scalar.activation(msk, pm, AF.Copy)  # PSUM -> SBUF

                  for c in range(C):
                      y = ypool.tile([128, 512], F32, tag="y")
                      V.scalar_tensor_tensor(y, x[c], Sb[:, 0:1], tmp, OP.mult, OP.add)
                      o = opool.tile([128, 512], F32, tag="o")
                      eng = nc.vector if c == 0 else nc.gpsimd
                      eng.tensor_tensor(o, y, msk, OP.mult)

                      nc.gpsimd.indirect_dma_start(
                          out=out.rearrange("(n u) -> n u", u=1),
                          out_offset=bass.IndirectOffsetOnAxis(
                              ap=offt[:, c : c + 1], axis=0
                          ),
                          in_=o[:, :],
                          in_offset=None,
                      )

    nc.compile()
    return nc


_CACHE: dict = {}


def _get_compiled(n_imgs: int, repeat: int = 1):
    key = (n_imgs, repeat)
    if key not in _CACHE:
        _CACHE[key] = _build_kernel(n_imgs, repeat)
    return _CACHE[key]


def _pack_core_inputs(imgs, br, sat, con, tx, ty, cx, cy):
    """imgs: [n,3,256,256] f32 and per-image params for ONE core shard."""
    n = imgs.shape[0]
    prm = np.zeros((8, n), np.float32)
    prm[0] = br.reshape(n)
    prm[1] = sat.reshape(n)
    prm[2] = con.reshape(n)
    prm[3] = tx.reshape(n).astype(np.float32)
    prm[4] = ty.reshape(n).astype(np.float32)
    prm[5] = cx.reshape(n).astype(np.float32)
    prm[6] = cy.reshape(n).astype(np.float32)
    return {
        "imgs": np.ascontiguousarray(imgs.reshape(n * C, PLANE), dtype=np.float32),
        "prmr": np.ascontiguousarray(prm.reshape(1, 8 * n)),
        "prmc": np.ascontiguousarray(prm.T),
    }


def kernel(imgs, br, sat, con, tx, ty, cx, cy, _trace=False, _trace_kwargs=None, _repeat=1):
    imgs = np.asarray(imgs, dtype=np.float32)
    br = np.asarray(br, dtype=np.float32)
    sat = np.asarray(sat, dtype=np.float32)
    con = np.asarray(con, dtype=np.float32)
    tx = np.asarray(tx, dtype=np.int32)
    ty = np.asarray(ty, dtype=np.int32)
    cx = np.asarray(cx, dtype=np.int32)
    cy = np.asarray(cy, dtype=np.int32)

    n = IMGS_PER_CORE
    nc = _get_compiled(n, _repeat)

    in_maps = []
    for k in range(N_CORES):
        sl = slice(k * n, (k + 1) * n)
        in_maps.append(
            _pack_core_inputs(
                imgs[sl], br[sl], sat[sl], con[sl], tx[sl], ty[sl], cx[sl], cy[sl]
            )
        )

    res = run_bass_kernel_spmd(
        nc,
        in_maps,
        core_ids=list(range(N_CORES)),
        trace=_trace,
        **(_trace_kwargs or {}),
    )

    out = np.empty((B_FULL, C, H, W), np.float32)
    for k in range(N_CORES):
        flat = np.asarray(res.results[k]["out"]).reshape(-1)
        for j in range(n):
            for c in range(C):
                base = (j * C + c) * SLOT + MARG
                out[k * n + j, c] = flat[base : base + PLANE].reshape(H, W)
    if _trace:
        kernel._last_results = res
    return out


kernel._last_results = None



# revision 14
# speedup vs baseline: 1097.4223x; 1097.4223x over previous
"""Trainium2 Bass kernel for nn_Augmenter (color jitter + translate + cutout).

Contract: kernel(**inputs) takes FULL unsharded numpy inputs
(imgs [128,3,256,256] f32, br/sat/con [128,1,1,1] f32,
 tx/ty/cx/cy [128,1,1] i32) and returns the FULL output [128,3,256,256] f32.

Internally: shard batch over 8 NeuronCores (16 images each), run one SPMD
Bass/Tile kernel via run_bass_kernel_spmd, reassemble on host.

Math (per image, derived from the reference):
  b' = br-0.5, s = 2*sat, c = con+0.5
  color:  x3 = A*x + Bp*MC + D
          A  = c*s
          Bp = c*(1-s)/3          (MC = sum over the 3 channels of x)
          D  = (1-c)*m0 + b'      (m0 = mean over all pixels+channels of x)
  translate by (txs,tys) = (tx-32, ty-32) with zero fill
  cutout: zero rows [max(0,cx-64), min(255,cx+63)] x cols [..cy..] in OUTPUT
          coordinates (cutout applied after translation)

Implementation (v3 -- fully static DMA, descriptors >= 2KB):
  * SBUF layout [128, 1536]: partition p holds input rows 2p, 2p+1 of all
    3 channels (c-major: chunk (c,b) at free 512c+256b).
  * Column shift (tys) + column-validity + cutout masking are ONE gpsimd
    local_scatter per plane: dst[p, idx[p,f]] = y[p, f]; idx encodes the
    shifted destination 256*b + (j - tys); invalid/cut pixels get a
    negative index (skipped) and local_scatter zero-fills the rest.
    idx (shared by the 3 channels) is built on the otherwise-idle
    TensorEngine as rank-1 matmuls into PSUM, then one ScalarE copy
    converts PSUM f32 -> int16. The cutout row range is tested against
    INPUT rows (k in [lo_x+txs, hi_x+txs]), so masking commutes with the
    row relabeling below.
  * Row shift (txs) is pure relabeling: the device stores partition p's
    block STATICALLY (its rows 2p,2p+1 hold the data for OUTPUT rows
    2p-txs, 2p+1-txs); the host shifts rows with numpy slicing during
    unpack (it knows tx) and zero-fills rows that scrolled out. No
    indirect DMA anywhere (real HW honors only one dynamic offset per
    partition, so scatter-stores are a poor fit anyway).
  * Output is stored as fp16 (rel tol is 2e-2; fp16 adds ~5e-4) halving
    store traffic: per-core HBM = 12.6MB read + 6.3MB write ~= 53us floor.
"""

import numpy as np

import concourse.bacc as bacc
import concourse.bass as bass
import concourse.mybir as mybir
import concourse.tile as tile
from concourse.bass_utils import run_bass_kernel_spmd

F32 = mybir.dt.float32
F16 = mybir.dt.float16
I16 = mybir.dt.int16
I32 = mybir.dt.int32
OP = mybir.AluOpType
AF = mybir.ActivationFunctionType

N_CORES = 8
B_FULL = 128
IMGS_PER_CORE = B_FULL // N_CORES  # 16
C, H, W = 3, 256, 256
PLANE = H * W  # 65536
BIG = 576.0                  # negative-index offset for masked pixels (fp16-exact)


def _build_kernel(n_imgs: int, repeat: int = 1):
    """Build + compile the per-core SPMD program.

    repeat > 1 wraps the whole per-image pipeline in a hardware For_i loop
    (identical work + writes each iteration) for wall-clock timing.
    """
    nc = bacc.Bacc(
        "TRN2",
        target_bir_lowering=False,
        debug=False,
        enable_asserts=False,
        num_devices=N_CORES,
    )
    n = n_imgs

    imgs_t = nc.dram_tensor("imgs", [n * C, PLANE], F32, kind="ExternalInput")
    prmr_t = nc.dram_tensor("prmr", [1, 8 * n], F32, kind="ExternalInput")
    prmc_t = nc.dram_tensor("prmc", [n, 8], F32, kind="ExternalInput")
    out_t = nc.dram_tensor("out", [n * 128, 1536], F16, kind="ExternalOutput")
    imgs = imgs_t.ap()
    prmr = prmr_t.ap()
    prmc = prmc_t.ap()
    out = out_t.ap()

    with tile.TileContext(nc) as tc:
        with (
            tc.tile_pool(name="const", bufs=1) as cpool,
            tc.tile_pool(name="xin", bufs=6) as xpool,
            tc.tile_pool(name="tt", bufs=3) as tpool,
            tc.tile_pool(name="mc", bufs=3) as mcpool,
            tc.tile_pool(name="tmp", bufs=4) as tmppool,
            tc.tile_pool(name="ix", bufs=4) as ixpool,
            tc.tile_pool(name="oo", bufs=3) as opool,
            tc.tile_pool(name="sm", bufs=12) as smpool,
            tc.tile_pool(name="st", bufs=6) as stpool,
            tc.tile_pool(name="ps", bufs=3, space="PSUM") as pspool,
            tc.tile_pool(name="pss", bufs=2, space="PSUM") as psspool,
        ):
            V = nc.vector
            G = nc.gpsimd

            # ---------------- one-time constants ----------------
            io_i = cpool.tile([n, 256], I32)
            G.iota(io_i, pattern=[[1, 256]], base=0, channel_multiplier=0)
            IO = cpool.tile([n, 256], F32)
            V.tensor_copy(IO, io_i)

            iom_i = cpool.tile([n, 512], I32)
            G.iota(iom_i, pattern=[[0, 2], [1, 256]], base=0, channel_multiplier=0)
            IOM = cpool.tile([n, 512], F32)
            V.tensor_copy(IOM, iom_i)

            io5_i = cpool.tile([n, 512], I32)
            G.iota(io5_i, pattern=[[1, 512]], base=0, channel_multiplier=0)
            IO512 = cpool.tile([n, 512], F32)
            V.tensor_copy(IO512, io5_i)

            ONES = cpool.tile([1, 128], F32)
            V.memset(ONES, 1.0)
            ONESH = cpool.tile([1, 128], F16)
            V.memset(ONESH, 1.0)
            ONE128 = cpool.tile([128, 1], F32)
            V.memset(ONE128, 1.0)

            # ---------------- parameter crunch ----------------
            Pr = cpool.tile([1, 8 * n], F32)
            nc.sync.dma_start(Pr, prmr)
            BRr, SATr, CONr = Pr[:, 0:n], Pr[:, n:2 * n], Pr[:, 2 * n:3 * n]

            Pc = cpool.tile([n, 8], F32)
            nc.sync.dma_start(Pc, prmc)
            TXc, TYc = Pc[:, 3:4], Pc[:, 4:5]
            CXc, CYc = Pc[:, 5:6], Pc[:, 6:7]

            # P3 row [1, 4n]: image i slots [4i..4i+4) = A, Bp, D, (unused)
            P3 = cpool.tile([1, 4 * n], F32)
            A_s = P3[:, 0:4 * n:4]
            Bp_s = P3[:, 1:4 * n:4]
            ROW = cpool.tile([1, 4 * n], F32)
            cf = ROW[:, 0:n]
            epp = ROW[:, n:2 * n]
            bpp = ROW[:, 2 * n:3 * n]
            rt = ROW[:, 3 * n:4 * n]

            V.tensor_scalar(cf, CONr, 1.0, 0.5, OP.mult, OP.add)
            # epp = (1 - cf)/196608 = (0.5 - con)/196608
            V.tensor_scalar(epp, CONr, -1.0 / 196608.0, 0.5 / 196608.0,
                            OP.mult, OP.add)
            V.tensor_scalar(bpp, BRr, 1.0, -0.5, OP.mult, OP.add)
            V.tensor_scalar(rt, SATr, 2.0, None, OP.mult)
            V.tensor_tensor(A_s, cf, rt, OP.mult)          # A = cf * 2sat
            V.tensor_tensor(rt, cf, A_s, OP.subtract)      # cf - A
            V.tensor_scalar(Bp_s, rt, 1.0 / 3.0, None, OP.mult)

            # ---------------- per-image vectors ST [n, 1024] (fp16) --------
            # [0:256)   rc   : input row 2p+b in cutout (rows lo_x+txs..hi_x+txs)
            # [256:512) ccsn : -BIG * (source col j lands in cutout cols)
            # [512:1024) w   : f - tys - BIG*(j - tys outside [0,256))
            ST = cpool.tile([n, 1024], F16)
            COL = cpool.tile([n, 8], F32)
            txs_c = COL[:, 0:1]
            tys_c = COL[:, 1:2]
            lo = COL[:, 2:3]
            hi = COL[:, 3:4]
            e1 = cpool.tile([n, 512], F32)
            wv = cpool.tile([n, 512], F32)

            V.tensor_scalar(txs_c, TXc, 1.0, -32.0, OP.mult, OP.add)
            V.tensor_scalar(tys_c, TYc, 1.0, -32.0, OP.mult, OP.add)

            # rc rows (input-row space)
            V.tensor_scalar(lo, CXc, 64.0, 0.0, OP.subtract, OP.max)
            V.tensor_tensor(lo, lo, txs_c, OP.add)
            V.tensor_scalar(hi, CXc, 63.0, 255.0, OP.add, OP.min)
            V.tensor_tensor(hi, hi, txs_c, OP.add)
            V.tensor_scalar(e1[:, 0:256], IO, hi, None, OP.is_le)
            V.scalar_tensor_tensor(ST[:, 0:256], IO, lo, e1[:, 0:256],
                                   OP.is_ge, OP.logical_and)

            # ccsn cols: -BIG * (lo_y+tys <= j <= hi_y+tys)
            V.tensor_scalar(lo, CYc, 64.0, 0.0, OP.subtract, OP.max)
            V.tensor_tensor(lo, lo, tys_c, OP.add)
            V.tensor_scalar(hi, CYc, 63.0, 255.0, OP.add, OP.min)
            V.tensor_tensor(hi, hi, tys_c, OP.add)
            V.tensor_scalar(e1[:, 0:256], IO, hi, None, OP.is_le)
            V.scalar_tensor_tensor(wv[:, 0:256], IO, lo, e1[:, 0:256],
                                   OP.is_ge, OP.logical_and)
            V.tensor_scalar(ST[:, 256:512], wv[:, 0:256], -BIG, None, OP.mult)

            # w: valid = (tys <= j <= 255+tys); w = (IO512 - tys - BIG) + BIG*valid
            V.tensor_scalar(hi, tys_c, 255.0, None, OP.add)
            V.tensor_scalar(e1, IOM, hi, None, OP.is_le)
            V.scalar_tensor_tensor(wv, IOM, tys_c, e1, OP.is_ge, OP.logical_and)
            V.tensor_scalar(e1, IO512, tys_c, BIG, OP.subtract, OP.subtract)
            V.scalar_tensor_tensor(ST[:, 512:1024], wv, BIG, e1, OP.mult, OP.add)

            # ---------------- per-image pipeline ----------------
            # Emitted as a 2-stage software pipeline with a 1-image skew:
            # front(i) = load + stats + index build; back(i) = y/scatter/store.
            def front(i):
                    x = xpool.tile([128, 1536], F32, tag="x")
                    for c in range(C):
                        nc.sync.dma_start(
                            x[:, 512 * c:512 * (c + 1)],
                            imgs[i * C + c].rearrange("(p f) -> p f", p=128),
                        )
                    st_ = stpool.tile([1, 1024], F16, tag="st")
                    nc.sync.dma_start(st_, ST[i:i + 1, :])

                    t = tpool.tile([128, 512], F32, tag="t")
                    V.tensor_tensor(t, x[:, 0:512], x[:, 512:1024], OP.add)
                    MC = mcpool.tile([128, 512], F32, tag="mc")
                    mcp = smpool.tile([128, 1], F32, tag="mcp")
                    V.scalar_tensor_tensor(MC, t, 1.0, x[:, 1024:1536],
                                           OP.mult, OP.add, accum_out=mcp)

                    # m0 sum across partitions on PE
                    m0ps = psspool.tile([1, 1], F32, tag="m0")
                    nc.tensor.matmul(m0ps, lhsT=mcp, rhs=ONE128,
                                     start=True, stop=True)
                    # D = epp*SUM + bpp -> P3 slot 4i+2  (tiny, DVE)
                    V.scalar_tensor_tensor(
                        P3[:, 4 * i + 2:4 * i + 3], m0ps[0:1, 0:1],
                        epp[:, i:i + 1], bpp[:, i:i + 1], OP.mult, OP.add)

                    # broadcast [A, Bp, D, .] to all partitions via PE
                    sbps = psspool.tile([128, 3], F32, tag="sbps")
                    nc.tensor.matmul(sbps, lhsT=ONES,
                                     rhs=P3[:, 4 * i:4 * i + 3],
                                     start=True, stop=True)
                    Sb = smpool.tile([128, 3], F32, tag="sb")
                    nc.scalar.activation(Sb, sbps, AF.Copy)

                    # scatter indices via PE rank-1s:
                    # pm = rc(2p+b) x ccsn  +  ones x w
                    pm = pspool.tile([128, 512], F32, tag="pm")
                    for b in range(2):
                        half = pm[:, 256 * b:256 * (b + 1)]
                        nc.tensor.matmul(half, lhsT=st_[:, b:256:2],
                                         rhs=st_[:, 256:512],
                                         start=True, stop=False)
                        nc.tensor.matmul(
                            half, lhsT=ONESH,
                            rhs=st_[:, 512 + 256 * b:768 + 256 * b],
                            start=False, stop=True)
                    idx = ixpool.tile([128, 512], I16, tag="idx")
                    nc.scalar.activation(idx, pm, AF.Copy)

                    # tmp = Bp*MC + D  (ScalarE)
                    tmp = tmppool.tile([128, 512], F32, tag="tmp")
                    nc.scalar.activation(tmp, MC, AF.Identity,
                                         bias=Sb[:, 2:3], scale=Sb[:, 1:2])
                    return x, Sb, tmp, idx

            def back(i, st):
                    x, Sb, tmp, idx = st
                    big = opool.tile([128, 1536], F16, tag="big")
                    for c in range(C):
                        y = smpool.tile([128, 512], F16, tag=f"y{c}")
                        V.scalar_tensor_tensor(y, x[:, 512 * c:512 * (c + 1)],
                                               Sb[:, 0:1], tmp,
                                               OP.mult, OP.add)
                        G.local_scatter(
                            big[:, 512 * c:512 * (c + 1)],
                            y, idx, channels=128, num_elems=512, num_idxs=512)

                    nc.scalar.dma_start(out[i * 128:(i + 1) * 128, :], big)

            def pipeline(skew=2):
                sts = []
                for i in range(n):
                    sts.append(front(i))
                    if i >= skew:
                        back(i - skew, sts[i - skew])
                for i in range(n - skew, n):
                    back(i, sts[i])

            if repeat > 1:
                with tc.For_i(0, repeat):
                    pipeline()
            else:
                pipeline()

    nc.compile()
    return nc


_CACHE: dict = {}


def _get_compiled(n_imgs: int, repeat: int = 1):
    key = (n_imgs, repeat)
    if key not in _CACHE:
        _CACHE[key] = _build_kernel(n_imgs, repeat)
    return _CACHE[key]


def _pack_core_inputs(imgs, br, sat, con, tx, ty, cx, cy):
    """imgs: [n,3,256,256] f32 and per-image params for ONE core shard."""
    n = imgs.shape[0]
    prm = np.zeros((8, n), np.float32)
    prm[0] = br.reshape(n)
    prm[1] = sat.reshape(n)
    prm[2] = con.reshape(n)
    prm[3] = tx.reshape(n).astype(np.float32)
    prm[4] = ty.reshape(n).astype(np.float32)
    prm[5] = cx.reshape(n).astype(np.float32)
    prm[6] = cy.reshape(n).astype(np.float32)
    return {
        "imgs": np.ascontiguousarray(imgs.reshape(n * C, PLANE), dtype=np.float32),
        "prmr": np.ascontiguousarray(prm.reshape(1, 8 * n)),
        "prmc": np.ascontiguousarray(prm.T),
    }


def kernel(imgs, br, sat, con, tx, ty, cx, cy, _trace=False, _trace_kwargs=None,
           _repeat=1):
    imgs = np.asarray(imgs, dtype=np.float32)
    br = np.asarray(br, dtype=np.float32)
    sat = np.asarray(sat, dtype=np.float32)
    con = np.asarray(con, dtype=np.float32)
    tx = np.asarray(tx, dtype=np.int32)
    ty = np.asarray(ty, dtype=np.int32)
    cx = np.asarray(cx, dtype=np.int32)
    cy = np.asarray(cy, dtype=np.int32)

    n = IMGS_PER_CORE
    nc = _get_compiled(n, _repeat)

    in_maps = []
    for k in range(N_CORES):
        sl = slice(k * n, (k + 1) * n)
        in_maps.append(
            _pack_core_inputs(
                imgs[sl], br[sl], sat[sl], con[sl], tx[sl], ty[sl], cx[sl], cy[sl]
            )
        )

    res = run_bass_kernel_spmd(
        nc,
        in_maps,
        core_ids=list(range(N_CORES)),
        trace=_trace,
        **(_trace_kwargs or {}),
    )

    txs_all = tx.reshape(B_FULL) - 32
    out = np.zeros((B_FULL, C, H, W), np.float32)
    for k in range(N_CORES):
        # [n*128, 1536] -> [n, 128, 3, 2, 256] -> (n, 3, 256, 256) in input rows
        arr = np.asarray(res.results[k]["out"]).reshape(n, 128, C, 2, W)
        arr = arr.transpose(0, 2, 1, 3, 4).reshape(n, C, H, W)
        for j in range(n):
            s = int(txs_all[k * n + j])
            # row k of arr holds the (colored, col-shifted, masked) data that
            # belongs at output row k - s; equivalently out[r] = arr[r + s]
            if s >= 0:
                out[k * n + j, :, 0:H - s, :] = arr[j, :, s:H, :]
            else:
                out[k * n + j, :, -s:H, :] = arr[j, :, 0:H + s, :]
    if _trace:
        kernel._last_results = res
    return out


kernel._last_results = None
